# revision 13
# baseline (speedup 1.0000x reference)
"""RNN-T JointNet kernel for 8 Trainium2 NeuronCores.

Math: out[b,t,u,:] = gelu_tanh(concat(enc[b,t], dec[b,u])) @ W_fc^T + b_fc
Since gelu is elementwise, gelu(concat(a,b)) = concat(gelu(a), gelu(b)), so
  out[b,t,u,:] = P_enc[b,t,:] + P_dec[b,u,:]
with P_enc = gelu(enc) @ W_fc[:, :512]^T          (small matmul)
     P_dec = gelu(dec) @ W_fc[:, 512:]^T + b_fc   (small matmul; bias folded
                                                   here so it rides the bct)
The dominant cost is streaming the (B,T,U,V) output to HBM. The output is
stored as bf16 (compute is already bf16; rel err ~4e-3) and upcast to f32 on
the host, halving HBM store traffic vs f32 (~20MB/core, ~55us at ~360GB/s).

Sharding: 8 cores = 2 batch-pairs x 4 u-quarters. Core c -> bp = c//4
(batches {0,1} or {2,3}), uq = c%4 with u range [25*uq, 25*uq+26) (26 rows,
1-row overlap between quarters; quarter q>0 contributes local rows 1..25).
Per-core row space: 600 (b,t) rows laid out as 5 chunks of 120 partitions
with the two batches INTERLEAVED across partitions: chunk c, partition p ->
batch 2*bp + p%2, t = c*60 + p//2. This avoids any ragged 44-row t-tail
(no gpsimd elementwise), keeps DVE lane utilization at 94%, and keeps SDMA
engine load balanced.

The host supplies gelu inputs PRE-TRANSPOSED (encT/decT, feature dim on
partitions) -- gelu commutes with transpose -- so the kernel needs no PE
transposes; ACT gelus encT/decT in place and the matmuls read them directly
as lhsT. A burst of tiny real matmuls (N=64 off a memset tile) during the
input-load wait warms the PE HAM clock-gate to 2.4GHz before P_dec arrives
(PE transposes would NOT warm it). Per-u pipeline: PE gathers the
per-partition addend P_dec[batch(p), u] + bias straight out of the [52,640]
pd tile into a [120,640] f32 PSUM tile via a K=52 matmul against a
host-loaded one-hot selector slice selU[:, 128u:128u+120] (no SBUF->SBUF
relayout DMA on the critical path); ACT copies it to a bf16 bct tile; one
DVE tensor_tensor (bf16 2x mode, ~1.8us) adds pe[120,5,640] (P_enc, bf16) +
bct (stride-0 broadcast over the 5 chunks) into the out tile j-slice.
Stores: one 768KB fully contiguous DMA per u, alternating HWDGE rings (two
in flight hides the ~2us HBM write-receipt latency), 5 ot bufs. Blocks 0-1
are computed and stored per chunk, interleaved, so stores start as soon as
P_enc chunk 0 is done and flow while chunks 1-4 are still being computed.
"""

import numpy as np

B, T, U = 4, 300, 101
D = 512
V = 640
UCORE = 26  # u rows per core (4 quarters x 25 + 1 overlap row)
NCORES = 8
UB = 2  # u rows per store block (26 = 13 * 2)
NBLK = UCORE // UB
NCH = 5  # row chunks of 120 partitions (600 = 2 batches x 300 t)
PCH = 120
NWARM = 30  # tiny matmuls to hold the PE HAM clock-gate open

LAST_RESULT = None  # BassKernelResults of the most recent run (for test.py)
RUN_KWARGS = {}  # extra kwargs test.py may inject (e.g. tmpdir for traces)

_cache = {}


def _build():
    import concourse.mybir as mybir
    from concourse import bacc
    from concourse.tile import TileContext

    f32 = mybir.dt.float32
    bf16 = mybir.dt.bfloat16
    AF = mybir.ActivationFunctionType

    nc = bacc.Bacc()
    # host pre-transposed: encT[p, dch, r] = enc feature d = dch*128+p of row
    # r = c*120 + q  (row -> batch q%2... see module docstring)
    encT_d = nc.dram_tensor("encT", [128, 4, NCH * PCH], bf16, kind="ExternalInput")
    # decT[p, dch, s*26+u] = dec feature d = dch*128+p of (batch s, u0+u)
    decT_d = nc.dram_tensor("decT", [128, 4, 2 * UCORE], bf16, kind="ExternalInput")
    # W_fc.T rows d = dch*128+p; wTd = dec half (cols 512..1023), wTe = enc half
    wTd_d = nc.dram_tensor("wTd", [128, 4, V], bf16, kind="ExternalInput")
    wTe_d = nc.dram_tensor("wTe", [128, 4, V], bf16, kind="ExternalInput")
    bias_d = nc.dram_tensor("bias", [1, V], bf16, kind="ExternalInput")
    # selU[r, 128u + p] = 1 iff r == (p%2)*26 + u: K=52 gather-broadcast masks
    selU_d = nc.dram_tensor("selU", [2 * UCORE, UCORE * 128], bf16, kind="ExternalInput")
    # output laid out exactly like the SBUF tiles so every store is one
    # fully contiguous DRAM write; host un-permutes + upcasts to f32.
    # outD[bi, p, j, c, v] = out[batch p%2, t = c*60 + p//2, u = UB*bi + j, v]
    outD = nc.dram_tensor("outD", [NBLK, PCH, UB, NCH, V], bf16, kind="ExternalOutput")

    vchunks = [(0, 512), (512, V - 512)]

    with TileContext(nc) as tc:
        with (
            tc.tile_pool(name="const", bufs=1) as constp,
            tc.tile_pool(name="persist", bufs=1) as persist,
            tc.tile_pool(name="outp", bufs=6) as outp,
            tc.tile_pool(name="bctp", bufs=8) as bctp,
            tc.tile_pool(name="psum", bufs=1, space="PSUM") as psum,
        ):
            # input loads. sync ring: decT, then wTd per d-chunk (P_dec runs
            # d-major and starts on chunk 0 before the rest land), then the
            # small consts. scalar ring carries ONLY the encT issue so the
            # ACT sequencer gets to its gelu table loads immediately. wTe per
            # d-chunk on the SWDGE ring overlaps both.
            decT = persist.tile([128, 4, 2 * UCORE], bf16, tag="decT")
            nc.sync.dma_start(decT[:, :, :], decT_d[:, :, :])
            wTd = persist.tile([128, 4, V], bf16, tag="wTd")
            nc.sync.dma_start(wTd[:, :, :], wTd_d[:, :, :])
            encT = persist.tile([128, 4, NCH * PCH], bf16, tag="encT")
            nc.scalar.dma_start(encT[:, :, :], encT_d[:, :, :])
            bias_sb = constp.tile([1, V], bf16)
            nc.sync.dma_start(bias_sb[:], bias_d[:])
            selU = constp.tile([2 * UCORE, UCORE * 128], bf16)
            nc.scalar.dma_start(selU[:, :], selU_d[:, :])
            ones1 = constp.tile([1, 128], bf16)
            nc.gpsimd.memset(ones1[:], 1.0)
            wTe = persist.tile([128, 4, V], bf16, tag="wTe")
            nc.gpsimd.dma_start(wTe[:, :, :], wTe_d[:, :, :])

            # warm the PE HAM clock-gate with tiny REAL matmuls while the
            # input DMAs land (the HAM watches matmul busy time; ~3.4us of
            # sustained activity lifts the PE from 1.2 to 2.4 GHz)
            for _ in range(NWARM):
                wm = psum.tile([1, 64], f32, tag="wm", bufs=2)
                nc.tensor.matmul(wm[0:1, :], ones1[0:1, 0:1], ones1[0:1, 0:64], start=True, stop=True)

            # gelu in the transposed layout (gelu commutes with transpose)
            gdecT = persist.tile([128, 4, 2 * UCORE], bf16, tag="gdecT")
            nc.scalar.activation(gdecT[:, :, :], decT[:, :, :], AF.Gelu_apprx_tanh)
            gencT = persist.tile([128, 4, NCH * PCH], bf16, tag="gencT")
            nc.scalar.activation(gencT[:, :, :], encT[:, :, :], AF.Gelu_apprx_tanh)

            # P_dec + bias -> pd [52, 640] bf16 (kept partition-major; the
            # per-u broadcast gathers rows straight from it via selU).
            # d-major so matmuls start as soon as wTd chunk 0 lands.
            pd_bf = persist.tile([2 * UCORE, V], bf16, tag="pd")
            ps = psum.tile([128, V], f32, tag="bc", bufs=3)
            for d in range(4):
                for v0, vn in vchunks:
                    nc.tensor.matmul(
                        ps[: 2 * UCORE, v0 : v0 + vn],
                        gdecT[:, d, : 2 * UCORE],
                        wTd[:, d, v0 : v0 + vn],
                        start=(d == 0),
                        stop=False,
                    )
            for v0, vn in vchunks:
                nc.tensor.matmul(
                    ps[: 2 * UCORE, v0 : v0 + vn],
                    ones1[0:1, : 2 * UCORE],
                    bias_sb[:1, v0 : v0 + vn],
                    start=False,
                    stop=True,
                )
            nc.vector.tensor_copy(pd_bf[: 2 * UCORE, :], ps[: 2 * UCORE, :])

            pe = persist.tile([128, NCH, V], bf16, tag="pe")

            def enc_chunk(c):
                ps = psum.tile([128, V], f32, tag="bc", bufs=3)
                for d in range(4):
                    for v0, vn in vchunks:
                        nc.tensor.matmul(
                            ps[:PCH, v0 : v0 + vn],
                            gencT[:, d, c * PCH : (c + 1) * PCH],
                            wTe[:, d, v0 : v0 + vn],
                            start=(d == 0),
                            stop=(d == 3),
                        )
                nc.scalar.copy(pe[:PCH, c, :], ps[:PCH, :])

            def bcast(u):
                ps = psum.tile([128, V], f32, tag="bc", bufs=3)
                for c0, cn in vchunks:
                    nc.tensor.matmul(
                        ps[:PCH, c0 : c0 + cn],
                        selU[:, u * 128 : u * 128 + PCH],
                        pd_bf[:, c0 : c0 + cn],
                        start=True,
                        stop=True,
                    )
                bct = bctp.tile([128, V], bf16, tag="bct", name="bct")
                nc.scalar.copy(bct[:PCH, :], ps[:PCH, :])
                return bct

            # P_enc chunk 0 first, then the broadcasts for blocks 0-3, then
            # the remaining chunks -- blocks 0-3 are computed per chunk below
            # so stores saturate the DMA engines from the moment chunk 0 +
            # pd are ready, while chunks 1-4 are still being computed.
            NEARLY = 4
            enc_chunk(0)
            bcts = [bcast(u) for u in range(2 * UB)]
            enc_chunk(1)
            bcts += [bcast(u) for u in range(2 * UB, NEARLY * UB)]
            ots = [
                outp.tile([128, UB, NCH, V], bf16, tag="ot", name=f"ot0{b}")
                for b in range(NEARLY)
            ]
            for c in range(NCH):
                if c > 1:
                    enc_chunk(c)
                for b in range(NEARLY):
                    for j in range(UB):
                        nc.vector.tensor_add(
                            ots[b][:PCH, j, c, :],
                            pe[:PCH, c, :],
                            bcts[UB * b + j][:PCH, :],
                        )
                    eng = nc.sync if c % 2 == b % 2 else nc.scalar
                    eng.dma_start(outD[b, :, :, c, :], ots[b][:PCH, :, c, :])

            # main loop: remaining blocks, one DVE add + one 768KB store per
            # u, stores alternating rings so two are always in flight
            for bi in range(NEARLY, NBLK):
                ot = outp.tile([128, UB, NCH, V], bf16, tag="ot", name="ot")
                for j in range(UB):
                    u = UB * bi + j
                    bct = bcast(u)
                    nc.vector.tensor_add(
                        ot[:PCH, j, :, :],
                        pe[:PCH, :, :],
                        bct[:PCH, :].unsqueeze(1).broadcast_to([PCH, NCH, V]),
                    )
                    eng = nc.sync if j % 2 == 0 else nc.scalar
                    eng.dma_start(outD[bi, :, j, :, :], ot[:PCH, j, :, :])

    nc.compile()
    return nc


def kernel(encoder_outputs, decoder_outputs, W_fc, b_fc):
    global LAST_RESULT
    import os

    import ml_dtypes
    from concourse.bass_utils import run_bass_kernel_spmd

    bf = ml_dtypes.bfloat16
    enc = np.asarray(encoder_outputs, dtype=np.float32)
    dec = np.asarray(decoder_outputs, dtype=np.float32)

    # row space per batch-pair: r = c*120 + 2*i + s -> batch 2*bp+s, t = c*60+i
    E = enc.reshape(2, 2, NCH, 60, D).transpose(0, 2, 3, 1, 4).reshape(2, 600, D)
    # encT[bp] = E[bp].T reshaped to [128, 4, 600]
    encT = np.ascontiguousarray(
        E.transpose(0, 2, 1).reshape(2, 4, 128, NCH * PCH).transpose(0, 2, 1, 3)
    ).astype(bf)

    wT = np.asarray(W_fc, dtype=np.float32).T  # (1024, 640)
    wT_tiled = wT.reshape(8, 128, V).transpose(1, 0, 2)  # [128, 8, 640]
    wTe = np.ascontiguousarray(wT_tiled[:, 0:4]).astype(bf)
    wTd = np.ascontiguousarray(wT_tiled[:, 4:8]).astype(bf)

    bias = np.asarray(b_fc, dtype=np.float32)[None, :].astype(bf)

    # selU[r, 128u + p] = 1 iff r == (p%2)*26 + u
    selU = np.zeros((2 * UCORE, UCORE * 128), dtype=bf)
    for u in range(UCORE):
        selU[u, u * 128 + 0 : (u + 1) * 128 : 2] = 1
        selU[UCORE + u, u * 128 + 1 : (u + 1) * 128 : 2] = 1

    if "nc" not in _cache:
        _cache["nc"] = _build()
    nc = _cache["nc"]

    in_maps = []
    for c in range(NCORES):
        bp, uq = c // 4, c % 4
        u0 = 25 * uq
        # Dc[s*26+u, :] = dec[2*bp+s, u0+u, :]; decT = Dc.T as [128, 4, 52]
        Dc = dec[2 * bp : 2 * bp + 2, u0 : u0 + UCORE].reshape(2 * UCORE, D)
        decT = np.ascontiguousarray(
            Dc.T.reshape(4, 128, 2 * UCORE).transpose(1, 0, 2)
        ).astype(bf)
        in_maps.append(
            {
                "encT": encT[bp],
                "decT": decT,
                "wTd": wTd,
                "wTe": wTe,
                "bias": bias,
                "selU": selU,
            }
        )

    res = run_bass_kernel_spmd(
        nc,
        in_maps,
        list(range(NCORES)),
        trace=bool(int(os.environ.get("KJ_TRACE", "0"))),
        **RUN_KWARGS,
    )
    LAST_RESULT = res

    out = np.empty((B, T, U, V), dtype=np.float32)
    for c in range(NCORES):
        bp, uq = c // 4, c % 4
        u0 = 25 * uq
        # outD (13,120,2,5,640): [bi,p,j,cc,v] -> s = p%2, t = cc*60 + p//2,
        # u = u0 + 2*bi + j
        arr = res.results[c]["outD"]
        # bf16 -> f32 exact upcast via bit shift (fast)
        f = (arr.view(np.uint16).astype(np.uint32) << 16).view(np.float32)
        cut = np.ascontiguousarray(
            f.reshape(NBLK, 60, 2, UB, NCH, V).transpose(2, 4, 1, 0, 3, 5)
        ).reshape(2, T, UCORE, V)
        lo = 0 if uq == 0 else 1  # quarter q>0: local row 0 is the overlap
        out[2 * bp, :, u0 + lo : u0 + UCORE, :] = cut[0, :, lo:, :]
        out[2 * bp + 1, :, u0 + lo : u0 + UCORE, :] = cut[1, :, lo:, :]
    return out


# revision 15
# speedup vs baseline: 1.0010x; 1.0010x over previous
"""RNN-T JointNet kernel for 8 Trainium2 NeuronCores.

Math: out[b,t,u,:] = gelu_tanh(concat(enc[b,t], dec[b,u])) @ W_fc^T + b_fc
Since gelu is elementwise, gelu(concat(a,b)) = concat(gelu(a), gelu(b)), so
  out[b,t,u,:] = P_enc[b,t,:] + P_dec[b,u,:]
with P_enc = gelu(enc) @ W_fc[:, :512]^T          (small matmul)
     P_dec = gelu(dec) @ W_fc[:, 512:]^T + b_fc   (small matmul; bias folded
                                                   here so it rides the bct)
The dominant cost is streaming the (B,T,U,V) output to HBM. The output is
stored as bf16 (compute is already bf16; rel err ~4e-3) and upcast to f32 on
the host, halving HBM store traffic vs f32 (~20MB/core, ~55us at ~360GB/s).

Sharding: 8 cores = 2 batch-pairs x 4 u-quarters. Core c -> bp = c//4
(batches {0,1} or {2,3}), uq = c%4 with u range [25*uq, 25*uq+26) (26 rows,
1-row overlap between quarters; quarter q>0 contributes local rows 1..25).
Per-core row space: 600 (b,t) rows laid out as 5 chunks of 120 partitions
with the two batches INTERLEAVED across partitions: chunk c, partition p ->
batch 2*bp + p%2, t = c*60 + p//2. This avoids any ragged 44-row t-tail
(no gpsimd elementwise), keeps DVE lane utilization at 94%, and keeps SDMA
engine load balanced.

The host supplies gelu inputs PRE-TRANSPOSED (encT/decT, feature dim on
partitions) -- gelu commutes with transpose -- so the kernel needs no PE
transposes; ACT gelus encT/decT in place and the matmuls read them directly
as lhsT. A burst of gapless N=128 matmuls during the input-load wait warms
the PE HAM clock-gate toward 2.4GHz before P_dec arrives. Per-u pipeline:
PE gathers the per-partition addend P_dec[batch(p), u] + bias straight out
of the [52,640] pd tile into a [120,640] f32 PSUM tile via a K=52 matmul
against a host-loaded one-hot selector slice selU[:, 128u:128u+120]; ACT
copies it to a bf16 bct tile; one DVE tensor_tensor (bf16 2x mode, ~1.8us)
adds pe[120,5,640] (P_enc, bf16) + bct (stride-0 broadcast over the 5
chunks) into the out tile j-slice, stored as one 768KB fully contiguous DMA
per u on alternating HWDGE rings (two in flight hides the ~2us HBM
write-receipt latency).

The first 8 u (blocks 0-3) instead go through a c-major early path so
stores saturate the DMA engines while P_enc chunks 1-4 are still being
computed: their bcts are materialized into two [120,4,640] group tiles and
one DVE op per (chunk, group) (pe chunk stride-0-broadcast over the 4 u)
fills a c-major otE tile whose chunk slice is stored immediately as a
614KB DMA with 5KB contiguous lines.
"""

import numpy as np

B, T, U = 4, 300, 101
D = 512
V = 640
UCORE = 26  # u rows per core (4 quarters x 25 + 1 overlap row)
NCORES = 8
UB = 2  # u rows per store block (26 = 13 * 2)
NBLK = UCORE // UB
NCH = 5  # row chunks of 120 partitions (600 = 2 batches x 300 t)
PCH = 120
NEARLY = 4  # leading blocks routed through the c-major early path
GEARLY = 2  # early groups (NEARLY*UB / 4)
NWARM = 20  # gapless N=128 matmuls to open the PE HAM clock-gate

LAST_RESULT = None  # BassKernelResults of the most recent run (for test.py)
RUN_KWARGS = {}  # extra kwargs test.py may inject (e.g. tmpdir for traces)

_cache = {}


def _build():
    import concourse.mybir as mybir
    from concourse import bacc
    from concourse.tile import TileContext

    f32 = mybir.dt.float32
    bf16 = mybir.dt.bfloat16
    AF = mybir.ActivationFunctionType

    nc = bacc.Bacc()
    # host pre-transposed: encT[p, dch, r] = enc feature d = dch*128+p of row
    # r = c*120 + q  (row -> batch q%2... see module docstring)
    encT_d = nc.dram_tensor("encT", [128, 4, NCH * PCH], bf16, kind="ExternalInput")
    # decT[p, dch, s*26+u] = dec feature d = dch*128+p of (batch s, u0+u)
    decT_d = nc.dram_tensor("decT", [128, 4, 2 * UCORE], bf16, kind="ExternalInput")
    # W_fc.T rows d = dch*128+p; wTd = dec half (cols 512..1023), wTe = enc half
    wTd_d = nc.dram_tensor("wTd", [128, 4, V], bf16, kind="ExternalInput")
    wTe_d = nc.dram_tensor("wTe", [128, 4, V], bf16, kind="ExternalInput")
    bias_d = nc.dram_tensor("bias", [1, V], bf16, kind="ExternalInput")
    # selU[r, 128u + p] = 1 iff r == (p%2)*26 + u: K=52 gather-broadcast masks
    selU_d = nc.dram_tensor("selU", [2 * UCORE, UCORE * 128], bf16, kind="ExternalInput")
    # outputs laid out exactly like the SBUF tiles so every store is one
    # fully contiguous DRAM write; host un-permutes + upcasts to f32.
    # outE[g, c, p, j4, v] = out[batch p%2, t = c*60 + p//2, u = 4g + j4, v]
    outE_d = nc.dram_tensor("outE", [GEARLY, NCH, PCH, 4, V], bf16, kind="ExternalOutput")
    # outD[bi, p, j, c, v] = out[.., u = NEARLY*UB + UB*bi + j, v]
    outD = nc.dram_tensor(
        "outD", [NBLK - NEARLY, PCH, UB, NCH, V], bf16, kind="ExternalOutput"
    )

    vchunks = [(0, 512), (512, V - 512)]

    with TileContext(nc) as tc:
        with (
            tc.tile_pool(name="const", bufs=1) as constp,
            tc.tile_pool(name="persist", bufs=1) as persist,
            tc.tile_pool(name="outp", bufs=4) as outp,
            tc.tile_pool(name="bctp", bufs=4) as bctp,
            tc.tile_pool(name="psum", bufs=1, space="PSUM") as psum,
        ):
            # input loads. sync ring: decT, wTd, bias (they gate the deepest
            # chain gelu -> P_dec -> bct). scalar ring: encT then selU (so
            # the ACT sequencer gets to its gelu table loads quickly). wTe on
            # the SWDGE ring overlaps both.
            decT = persist.tile([128, 4, 2 * UCORE], bf16, tag="decT")
            nc.sync.dma_start(decT[:, :, :], decT_d[:, :, :])
            wTd = persist.tile([128, 4, V], bf16, tag="wTd")
            nc.sync.dma_start(wTd[:, :, :], wTd_d[:, :, :])
            encT = persist.tile([128, 4, NCH * PCH], bf16, tag="encT")
            nc.scalar.dma_start(encT[:, :, :], encT_d[:, :, :])
            bias_sb = constp.tile([1, V], bf16)
            nc.sync.dma_start(bias_sb[:], bias_d[:])
            selU = constp.tile([2 * UCORE, UCORE * 128], bf16)
            nc.scalar.dma_start(selU[:, :], selU_d[:, :])
            ones1 = constp.tile([1, 128], bf16)
            nc.gpsimd.memset(ones1[:], 1.0)
            wTe = persist.tile([128, 4, V], bf16, tag="wTe")
            nc.gpsimd.dma_start(wTe[:, :, :], wTe_d[:, :, :])

            # warm the PE HAM clock-gate with gapless tiny matmuls while the
            # input DMAs land (~3.4us of sustained activity lifts the PE from
            # 1.2 to 2.4 GHz; the 4-deep psum rotation hides the WAW waits)
            for _ in range(NWARM):
                wm = psum.tile([128, V], f32, tag="bc", bufs=4)
                nc.tensor.matmul(wm[0:1, 0:128], ones1[0:1, 0:1], ones1[0:1, :], start=True, stop=True)

            # gelu in the transposed layout (gelu commutes with transpose)
            gdecT = persist.tile([128, 4, 2 * UCORE], bf16, tag="gdecT")
            nc.scalar.activation(gdecT[:, :, :], decT[:, :, :], AF.Gelu_apprx_tanh)
            gencT = persist.tile([128, 4, NCH * PCH], bf16, tag="gencT")
            nc.scalar.activation(gencT[:, :, :], encT[:, :, :], AF.Gelu_apprx_tanh)

            # P_dec + bias -> pd [52, 640] bf16 (kept partition-major; the
            # per-u broadcast gathers rows straight from it via selU).
            # d-major so matmuls start as soon as wTd lands.
            pd_bf = persist.tile([2 * UCORE, V], bf16, tag="pd")
            ps = psum.tile([128, V], f32, tag="bc", bufs=4)
            for d in range(4):
                for v0, vn in vchunks:
                    nc.tensor.matmul(
                        ps[: 2 * UCORE, v0 : v0 + vn],
                        gdecT[:, d, : 2 * UCORE],
                        wTd[:, d, v0 : v0 + vn],
                        start=(d == 0),
                        stop=False,
                    )
            for v0, vn in vchunks:
                nc.tensor.matmul(
                    ps[: 2 * UCORE, v0 : v0 + vn],
                    ones1[0:1, : 2 * UCORE],
                    bias_sb[:1, v0 : v0 + vn],
                    start=False,
                    stop=True,
                )
            nc.vector.tensor_copy(pd_bf[: 2 * UCORE, :], ps[: 2 * UCORE, :])

            pe = persist.tile([128, NCH, V], bf16, tag="pe")

            def enc_chunk(c):
                ps = psum.tile([128, V], f32, tag="bc", bufs=4)
                for d in range(4):
                    for v0, vn in vchunks:
                        nc.tensor.matmul(
                            ps[:PCH, v0 : v0 + vn],
                            gencT[:, d, c * PCH : (c + 1) * PCH],
                            wTe[:, d, v0 : v0 + vn],
                            start=(d == 0),
                            stop=(d == 3),
                        )
                nc.scalar.copy(pe[:PCH, c, :], ps[:PCH, :])

            def bcast_ps(u):
                ps = psum.tile([128, V], f32, tag="bc", bufs=4)
                for c0, cn in vchunks:
                    nc.tensor.matmul(
                        ps[:PCH, c0 : c0 + cn],
                        selU[:, u * 128 : u * 128 + PCH],
                        pd_bf[:, c0 : c0 + cn],
                        start=True,
                        stop=True,
                    )
                return ps

            # early path: u 0..7 in two groups of 4. Each group's bcts are
            # materialized side by side so one DVE op covers (chunk, group)
            # and each chunk slice is stored the moment it is ready.
            bctg = [
                persist.tile([128, 4, V], bf16, tag=f"bctg{g}", name=f"bctg{g}")
                for g in range(GEARLY)
            ]
            otE = [
                persist.tile([128, NCH, 4, V], bf16, tag=f"otE{g}", name=f"otE{g}")
                for g in range(GEARLY)
            ]
            enc_chunk(0)
            for g in range(GEARLY):
                for j4 in range(4):
                    ps = bcast_ps(4 * g + j4)
                    nc.scalar.copy(bctg[g][:PCH, j4, :], ps[:PCH, :])
                if g == 0:
                    enc_chunk(1)
            for c in range(NCH):
                if c > 1:
                    enc_chunk(c)
                for g in range(GEARLY):
                    nc.vector.tensor_add(
                        otE[g][:PCH, c, :, :],
                        pe[:PCH, c, :].unsqueeze(1).broadcast_to([PCH, 4, V]),
                        bctg[g][:PCH, :, :],
                    )
                    eng = nc.sync if (c + g) % 2 == 0 else nc.scalar
                    eng.dma_start(outE_d[g, c, :, :, :], otE[g][:PCH, c, :, :])

            # main loop: blocks 4..12, one DVE add + one 768KB store per u,
            # stores alternating rings so two are always in flight
            for bi in range(NEARLY, NBLK):
                ot = outp.tile([128, UB, NCH, V], bf16, tag="ot", name="ot")
                for j in range(UB):
                    u = UB * bi + j
                    ps = bcast_ps(u)
                    bct = bctp.tile([128, V], bf16, tag="bct", name="bct")
                    nc.scalar.copy(bct[:PCH, :], ps[:PCH, :])
                    nc.vector.tensor_add(
                        ot[:PCH, j, :, :],
                        pe[:PCH, :, :],
                        bct[:PCH, :].unsqueeze(1).broadcast_to([PCH, NCH, V]),
                    )
                    eng = nc.sync if j % 2 == 0 else nc.scalar
                    eng.dma_start(outD[bi - NEARLY, :, j, :, :], ot[:PCH, j, :, :])

    nc.compile()
    return nc


def kernel(encoder_outputs, decoder_outputs, W_fc, b_fc):
    global LAST_RESULT
    import os

    import ml_dtypes
    from concourse.bass_utils import run_bass_kernel_spmd

    bf = ml_dtypes.bfloat16
    enc = np.asarray(encoder_outputs, dtype=np.float32)
    dec = np.asarray(decoder_outputs, dtype=np.float32)

    # row space per batch-pair: r = c*120 + 2*i + s -> batch 2*bp+s, t = c*60+i
    E = enc.reshape(2, 2, NCH, 60, D).transpose(0, 2, 3, 1, 4).reshape(2, 600, D)
    # encT[bp] = E[bp].T reshaped to [128, 4, 600]
    encT = np.ascontiguousarray(
        E.transpose(0, 2, 1).reshape(2, 4, 128, NCH * PCH).transpose(0, 2, 1, 3)
    ).astype(bf)

    wT = np.asarray(W_fc, dtype=np.float32).T  # (1024, 640)
    wT_tiled = wT.reshape(8, 128, V).transpose(1, 0, 2)  # [128, 8, 640]
    wTe = np.ascontiguousarray(wT_tiled[:, 0:4]).astype(bf)
    wTd = np.ascontiguousarray(wT_tiled[:, 4:8]).astype(bf)

    bias = np.asarray(b_fc, dtype=np.float32)[None, :].astype(bf)

    # selU[r, 128u + p] = 1 iff r == (p%2)*26 + u
    selU = np.zeros((2 * UCORE, UCORE * 128), dtype=bf)
    for u in range(UCORE):
        selU[u, u * 128 + 0 : (u + 1) * 128 : 2] = 1
        selU[UCORE + u, u * 128 + 1 : (u + 1) * 128 : 2] = 1

    if "nc" not in _cache:
        _cache["nc"] = _build()
    nc = _cache["nc"]

    in_maps = []
    for c in range(NCORES):
        bp, uq = c // 4, c % 4
        u0 = 25 * uq
        # Dc[s*26+u, :] = dec[2*bp+s, u0+u, :]; decT = Dc.T as [128, 4, 52]
        Dc = dec[2 * bp : 2 * bp + 2, u0 : u0 + UCORE].reshape(2 * UCORE, D)
        decT = np.ascontiguousarray(
            Dc.T.reshape(4, 128, 2 * UCORE).transpose(1, 0, 2)
        ).astype(bf)
        in_maps.append(
            {
                "encT": encT[bp],
                "decT": decT,
                "wTd": wTd,
                "wTe": wTe,
                "bias": bias,
                "selU": selU,
            }
        )

    res = run_bass_kernel_spmd(
        nc,
        in_maps,
        list(range(NCORES)),
        trace=bool(int(os.environ.get("KJ_TRACE", "0"))),
        **RUN_KWARGS,
    )
    LAST_RESULT = res

    out = np.empty((B, T, U, V), dtype=np.float32)
    for c in range(NCORES):
        bp, uq = c // 4, c % 4
        u0 = 25 * uq
        # bf16 -> f32 exact upcast via bit shift (fast)
        aE = res.results[c]["outE"]  # [2, 5, 120, 4, 640] -> u 0..7
        aD = res.results[c]["outD"]  # [9, 120, 2, 5, 640] -> u 8..25
        fE = (aE.view(np.uint16).astype(np.uint32) << 16).view(np.float32)
        fD = (aD.view(np.uint16).astype(np.uint32) << 16).view(np.float32)
        # outE [g, cc, p, j4, v]: p -> (i, s); -> [s, cc, i, g, j4, v]
        cutE = np.ascontiguousarray(
            fE.reshape(GEARLY, NCH, 60, 2, 4, V).transpose(3, 1, 2, 0, 4, 5)
        ).reshape(2, T, NEARLY * UB, V)
        # outD [bi, p, j, cc, v] -> [s, cc, i, bi, j, v]
        cutD = np.ascontiguousarray(
            fD.reshape(NBLK - NEARLY, 60, 2, UB, NCH, V).transpose(2, 4, 1, 0, 3, 5)
        ).reshape(2, T, UCORE - NEARLY * UB, V)
        cut = np.concatenate([cutE, cutD], axis=2)  # (2, 300, 26, 640)
        lo = 0 if uq == 0 else 1  # quarter q>0: local row 0 is the overlap
        out[2 * bp, :, u0 + lo : u0 + UCORE, :] = cut[0, :, lo:, :]
        out[2 * bp + 1, :, u0 + lo : u0 + UCORE, :] = cut[1, :, lo:, :]
    return out


# revision 22
# speedup vs baseline: 1.0280x; 1.0269x over previous
"""RNN-T JointNet kernel for 8 Trainium2 NeuronCores.

Math: out[b,t,u,:] = gelu_tanh(concat(enc[b,t], dec[b,u])) @ W_fc^T + b_fc
Since gelu is elementwise, gelu(concat(a,b)) = concat(gelu(a), gelu(b)), so
  out[b,t,u,:] = P_enc[b,t,:] + P_dec[b,u,:]
with P_enc = gelu(enc) @ W_fc[:, :512]^T          (small matmul)
     P_dec = gelu(dec) @ W_fc[:, 512:]^T + b_fc   (small matmul; bias folded
                                                   here so it rides the bct)
The dominant cost is streaming the (B,T,U,V) output to HBM. The output is
stored as bf16 (compute is already bf16; rel err ~4e-3) and upcast to f32 on
the host, halving HBM store traffic vs f32 (~20MB/core, ~55us at ~360GB/s).

Sharding: 8 cores = 2 batch-pairs x 4 u-quarters. Core c -> bp = c//4
(batches {0,1} or {2,3}), uq = c%4 with u range [25*uq, 25*uq+26) (26 rows,
1-row overlap between quarters; quarter q>0 contributes local rows 1..25).
Per-core row space: 600 (b,t) rows laid out as 5 chunks of 120 partitions
with the two batches INTERLEAVED across partitions: chunk c, partition p ->
batch 2*bp + p%2, t = c*60 + p//2. This avoids any ragged 44-row t-tail
(no gpsimd elementwise), keeps DVE lane utilization at 94%, and keeps SDMA
engine load balanced.

The host supplies gelu inputs PRE-TRANSPOSED (encT/decT, feature dim on
partitions) -- gelu commutes with transpose -- so the kernel needs no PE
transposes; ACT gelus encT/decT in place and the matmuls read them directly
as lhsT. A burst of gapless N=128 matmuls during the input-load wait warms
the PE HAM clock-gate toward 2.4GHz before P_dec arrives. Per-u pipeline:
PE gathers the per-partition addend P_dec[batch(p), u] + bias straight out
of the [52,640] pd tile into a [120,640] f32 PSUM tile via a K=52 matmul
against a host-loaded one-hot selector slice selU[:, 128u:128u+120]; ACT
copies it to a bf16 bct tile; one DVE tensor_tensor (bf16 2x mode, ~1.8us)
adds pe[120,5,640] (P_enc, bf16) + bct (stride-0 broadcast over the 5
chunks) into the out tile j-slice, stored as one 768KB fully contiguous DMA
per u on alternating HWDGE rings (two in flight hides the ~2us HBM
write-receipt latency).

The first 4 u (blocks 0-1) instead go through a c-major early path so
stores start as soon as P_enc chunk 0 + pd are ready and flow while chunks
1-4 are still being computed: per-(chunk, u) DVE adds fill c-major otE
tiles whose chunk slices are stored immediately (307KB DMAs, 2560B
contiguous lines).
"""

import numpy as np

B, T, U = 4, 300, 101
D = 512
V = 640
UCORE = 26  # u rows per core (4 quarters x 25 + 1 overlap row)
NCORES = 8
UB = 2  # u rows per store block (26 = 13 * 2)
NBLK = UCORE // UB
NCH = 5  # row chunks of 120 partitions (600 = 2 batches x 300 t)
PCH = 120
NEARLY = 2  # leading blocks routed through the c-major early path
NWARM = 20  # gapless N=128 matmuls to open the PE HAM clock-gate

LAST_RESULT = None  # BassKernelResults of the most recent run (for test.py)
RUN_KWARGS = {}  # extra kwargs test.py may inject (e.g. tmpdir for traces)

_cache = {}


def _build():
    import concourse.mybir as mybir
    from concourse import bacc
    from concourse.tile import TileContext

    f32 = mybir.dt.float32
    bf16 = mybir.dt.bfloat16
    AF = mybir.ActivationFunctionType

    nc = bacc.Bacc()
    # host pre-transposed: encT[p, dch, r] = enc feature d = dch*128+p of row
    # r = c*120 + q  (row -> batch q%2... see module docstring)
    encT_d = nc.dram_tensor("encT", [128, 4, NCH * PCH], bf16, kind="ExternalInput")
    # decT[p, dch, s*26+u] = dec feature d = dch*128+p of (batch s, u0+u)
    decT_d = nc.dram_tensor("decT", [128, 4, 2 * UCORE], bf16, kind="ExternalInput")
    # W_fc.T rows d = dch*128+p; wTd = dec half (cols 512..1023), wTe = enc half
    wTd_d = nc.dram_tensor("wTd", [128, 4, V], bf16, kind="ExternalInput")
    wTe_d = nc.dram_tensor("wTe", [128, 4, V], bf16, kind="ExternalInput")
    bias_d = nc.dram_tensor("bias", [1, V], bf16, kind="ExternalInput")
    # selU[r, 128u + p] = 1 iff r == (p%2)*26 + u: K=52 gather-broadcast masks
    selU_d = nc.dram_tensor("selU", [2 * UCORE, UCORE * 128], bf16, kind="ExternalInput")
    # outputs laid out exactly like the SBUF tiles so every store is one
    # fully contiguous DRAM write; host un-permutes + upcasts to f32.
    # outE[b, c, p, j, v] = out[batch p%2, t = c*60 + p//2, u = UB*b + j, v]
    outE_d = nc.dram_tensor("outE", [NEARLY, NCH, PCH, UB, V], bf16, kind="ExternalOutput")
    # outD[bi, p, j, c, v] = out[.., u = NEARLY*UB + UB*bi + j, v]
    outD = nc.dram_tensor(
        "outD", [NBLK - NEARLY, PCH, UB, NCH, V], bf16, kind="ExternalOutput"
    )

    vchunks = [(0, 512), (512, V - 512)]

    with TileContext(nc) as tc:
        with (
            tc.tile_pool(name="const", bufs=1) as constp,
            tc.tile_pool(name="persist", bufs=1) as persist,
            tc.tile_pool(name="outp", bufs=5) as outp,
            tc.tile_pool(name="bctp", bufs=6) as bctp,
            tc.tile_pool(name="psum", bufs=1, space="PSUM") as psum,
        ):
            # input loads. sync ring: decT, wTd, bias (they gate the deepest
            # chain gelu -> P_dec -> bct). scalar ring: encT then selU (so
            # the ACT sequencer gets to its gelu table loads quickly). wTe on
            # the SWDGE ring overlaps both.
            decT = persist.tile([128, 4, 2 * UCORE], bf16, tag="decT")
            nc.sync.dma_start(decT[:, :, :], decT_d[:, :, :])
            wTd = persist.tile([128, 4, V], bf16, tag="wTd")
            for dch in range(4):
                nc.sync.dma_start(wTd[:, dch, :], wTd_d[:, dch, :])
            encT = persist.tile([128, 4, NCH * PCH], bf16, tag="encT")
            nc.scalar.dma_start(encT[:, :, :], encT_d[:, :, :])
            bias_sb = constp.tile([1, V], bf16)
            nc.sync.dma_start(bias_sb[:], bias_d[:])
            selU = constp.tile([2 * UCORE, UCORE * 128], bf16)
            nc.scalar.dma_start(selU[:, :], selU_d[:, :])
            ones1 = constp.tile([1, 128], bf16)
            nc.gpsimd.memset(ones1[:], 1.0)
            wTe = persist.tile([128, 4, V], bf16, tag="wTe")
            for dch in range(4):
                nc.gpsimd.dma_start(wTe[:, dch, :], wTe_d[:, dch, :])

            # warm the PE HAM clock-gate with gapless tiny matmuls while the
            # input DMAs land (~3.4us of sustained activity lifts the PE from
            # 1.2 to 2.4 GHz; the 4-deep psum rotation hides the WAW waits)
            for _ in range(NWARM):
                wm = psum.tile([128, V], f32, tag="bc", bufs=4)
                nc.tensor.matmul(wm[0:1, 0:128], ones1[0:1, 0:1], ones1[0:1, :], start=True, stop=True)

            # gelu in the transposed layout (gelu commutes with transpose)
            gdecT = persist.tile([128, 4, 2 * UCORE], bf16, tag="gdecT")
            nc.scalar.activation(gdecT[:, :, :], decT[:, :, :], AF.Gelu_apprx_tanh)
            gencT = persist.tile([128, 4, NCH * PCH], bf16, tag="gencT")
            nc.scalar.activation(gencT[:, :, :], encT[:, :, :], AF.Gelu_apprx_tanh)

            # P_dec + bias -> pd [52, 640] bf16 (kept partition-major; the
            # per-u broadcast gathers rows straight from it via selU).
            # d-major so matmuls start as soon as wTd lands.
            pd_bf = persist.tile([2 * UCORE, V], bf16, tag="pd")
            ps = psum.tile([128, V], f32, tag="bc", bufs=4)
            for d in range(4):
                for v0, vn in vchunks:
                    nc.tensor.matmul(
                        ps[: 2 * UCORE, v0 : v0 + vn],
                        gdecT[:, d, : 2 * UCORE],
                        wTd[:, d, v0 : v0 + vn],
                        start=(d == 0),
                        stop=False,
                    )
            for v0, vn in vchunks:
                nc.tensor.matmul(
                    ps[: 2 * UCORE, v0 : v0 + vn],
                    ones1[0:1, : 2 * UCORE],
                    bias_sb[:1, v0 : v0 + vn],
                    start=False,
                    stop=True,
                )
            nc.vector.tensor_copy(pd_bf[: 2 * UCORE, :], ps[: 2 * UCORE, :])

            pe = persist.tile([128, NCH, V], bf16, tag="pe")

            def enc_chunk(c):
                ps = psum.tile([128, V], f32, tag="bc", bufs=4)
                for d in range(4):
                    for v0, vn in vchunks:
                        nc.tensor.matmul(
                            ps[:PCH, v0 : v0 + vn],
                            gencT[:, d, c * PCH : (c + 1) * PCH],
                            wTe[:, d, v0 : v0 + vn],
                            start=(d == 0),
                            stop=(d == 3),
                        )
                nc.scalar.copy(pe[:PCH, c, :], ps[:PCH, :])

            def bcast_ps(u):
                ps = psum.tile([128, V], f32, tag="bc", bufs=4)
                for c0, cn in vchunks:
                    nc.tensor.matmul(
                        ps[:PCH, c0 : c0 + cn],
                        selU[:, u * 128 : u * 128 + PCH],
                        pd_bf[:, c0 : c0 + cn],
                        start=True,
                        stop=True,
                    )
                return ps

            # early path: blocks 0-1 (u 0..3) computed per chunk in c-major
            # ot tiles, each chunk slice stored (2560B contiguous lines) the
            # moment its DVE add lands -- stores flow while P_enc chunks 1-4
            # are still being computed.
            enc_chunk(0)
            bcts = []
            for u in range(NEARLY * UB):
                ps = bcast_ps(u)
                bct = bctp.tile([128, V], bf16, tag="bct", name="bct")
                nc.scalar.copy(bct[:PCH, :], ps[:PCH, :])
                bcts.append(bct)
            otE = [
                persist.tile([128, NCH, UB, V], bf16, tag=f"otE{b}", name=f"otE{b}")
                for b in range(NEARLY)
            ]
            for c in range(NCH):
                if c > 0:
                    enc_chunk(c)
                for b in range(NEARLY):
                    for j in range(UB):
                        nc.vector.tensor_add(
                            otE[b][:PCH, c, j, :],
                            pe[:PCH, c, :],
                            bcts[UB * b + j][:PCH, :],
                        )
                    eng = nc.sync if (c + b) % 2 == 0 else nc.scalar
                    eng.dma_start(outE_d[b, c, :, :, :], otE[b][:PCH, c, :, :])

            # main loop: blocks 4..12, one DVE add + one 768KB store per u,
            # stores alternating rings so two are always in flight
            for bi in range(NEARLY, NBLK):
                ot = outp.tile([128, UB, NCH, V], bf16, tag="ot", name="ot")
                for j in range(UB):
                    u = UB * bi + j
                    ps = bcast_ps(u)
                    bct = bctp.tile([128, V], bf16, tag="bct", name="bct")
                    nc.scalar.copy(bct[:PCH, :], ps[:PCH, :])
                    nc.vector.tensor_add(
                        ot[:PCH, j, :, :],
                        pe[:PCH, :, :],
                        bct[:PCH, :].unsqueeze(1).broadcast_to([PCH, NCH, V]),
                    )
                    eng = nc.sync if j % 2 == 0 else nc.scalar
                    eng.dma_start(outD[bi - NEARLY, :, j, :, :], ot[:PCH, j, :, :])

    nc.compile()
    return nc


def kernel(encoder_outputs, decoder_outputs, W_fc, b_fc):
    global LAST_RESULT
    import os

    import ml_dtypes
    from concourse.bass_utils import run_bass_kernel_spmd

    bf = ml_dtypes.bfloat16
    enc = np.asarray(encoder_outputs, dtype=np.float32)
    dec = np.asarray(decoder_outputs, dtype=np.float32)

    # row space per batch-pair: r = c*120 + 2*i + s -> batch 2*bp+s, t = c*60+i
    E = enc.reshape(2, 2, NCH, 60, D).transpose(0, 2, 3, 1, 4).reshape(2, 600, D)
    # encT[bp] = E[bp].T reshaped to [128, 4, 600]
    encT = np.ascontiguousarray(
        E.transpose(0, 2, 1).reshape(2, 4, 128, NCH * PCH).transpose(0, 2, 1, 3)
    ).astype(bf)

    wT = np.asarray(W_fc, dtype=np.float32).T  # (1024, 640)
    wT_tiled = wT.reshape(8, 128, V).transpose(1, 0, 2)  # [128, 8, 640]
    wTe = np.ascontiguousarray(wT_tiled[:, 0:4]).astype(bf)
    wTd = np.ascontiguousarray(wT_tiled[:, 4:8]).astype(bf)

    bias = np.asarray(b_fc, dtype=np.float32)[None, :].astype(bf)

    # selU[r, 128u + p] = 1 iff r == (p%2)*26 + u
    selU = np.zeros((2 * UCORE, UCORE * 128), dtype=bf)
    for u in range(UCORE):
        selU[u, u * 128 + 0 : (u + 1) * 128 : 2] = 1
        selU[UCORE + u, u * 128 + 1 : (u + 1) * 128 : 2] = 1

    if "nc" not in _cache:
        _cache["nc"] = _build()
    nc = _cache["nc"]

    in_maps = []
    for c in range(NCORES):
        bp, uq = c // 4, c % 4
        u0 = 25 * uq
        # Dc[s*26+u, :] = dec[2*bp+s, u0+u, :]; decT = Dc.T as [128, 4, 52]
        Dc = dec[2 * bp : 2 * bp + 2, u0 : u0 + UCORE].reshape(2 * UCORE, D)
        decT = np.ascontiguousarray(
            Dc.T.reshape(4, 128, 2 * UCORE).transpose(1, 0, 2)
        ).astype(bf)
        in_maps.append(
            {
                "encT": encT[bp],
                "decT": decT,
                "wTd": wTd,
                "wTe": wTe,
                "bias": bias,
                "selU": selU,
            }
        )

    res = run_bass_kernel_spmd(
        nc,
        in_maps,
        list(range(NCORES)),
        trace=bool(int(os.environ.get("KJ_TRACE", "0"))),
        **RUN_KWARGS,
    )
    LAST_RESULT = res

    out = np.empty((B, T, U, V), dtype=np.float32)
    for c in range(NCORES):
        bp, uq = c // 4, c % 4
        u0 = 25 * uq
        # bf16 -> f32 exact upcast via bit shift (fast)
        aE = res.results[c]["outE"]  # [2, 5, 120, 2, 640] -> u 0..3
        aD = res.results[c]["outD"]  # [11, 120, 2, 5, 640] -> u 4..25
        fE = (aE.view(np.uint16).astype(np.uint32) << 16).view(np.float32)
        fD = (aD.view(np.uint16).astype(np.uint32) << 16).view(np.float32)
        # outE [b, cc, p, j, v]: p -> (i, s); -> [s, cc, i, b, j, v]
        cutE = np.ascontiguousarray(
            fE.reshape(NEARLY, NCH, 60, 2, UB, V).transpose(3, 1, 2, 0, 4, 5)
        ).reshape(2, T, NEARLY * UB, V)
        # outD [bi, p, j, cc, v] -> [s, cc, i, bi, j, v]
        cutD = np.ascontiguousarray(
            fD.reshape(NBLK - NEARLY, 60, 2, UB, NCH, V).transpose(2, 4, 1, 0, 3, 5)
        ).reshape(2, T, UCORE - NEARLY * UB, V)
        cut = np.concatenate([cutE, cutD], axis=2)  # (2, 300, 26, 640)
        lo = 0 if uq == 0 else 1  # quarter q>0: local row 0 is the overlap
        out[2 * bp, :, u0 + lo : u0 + UCORE, :] = cut[0, :, lo:, :]
        out[2 * bp + 1, :, u0 + lo : u0 + UCORE, :] = cut[1, :, lo:, :]
    return out


# revision 24
# speedup vs baseline: 1.0483x; 1.0198x over previous
"""RNN-T JointNet kernel for 8 Trainium2 NeuronCores.

Math: out[b,t,u,:] = gelu_tanh(concat(enc[b,t], dec[b,u])) @ W_fc^T + b_fc
Since gelu is elementwise, gelu(concat(a,b)) = concat(gelu(a), gelu(b)), so
  out[b,t,u,:] = P_enc[b,t,:] + P_dec[b,u,:]
with P_enc = gelu(enc) @ W_fc[:, :512]^T          (small matmul)
     P_dec = gelu(dec) @ W_fc[:, 512:]^T + b_fc   (small matmul; bias folded
                                                   here so it rides the bct)
The dominant cost is streaming the (B,T,U,V) output to HBM. The output is
stored as bf16 (compute is already bf16; rel err ~4e-3) and upcast to f32 on
the host, halving HBM store traffic vs f32 (~20MB/core, ~55us at ~360GB/s).

Sharding: 8 cores = 2 batch-pairs x 4 u-quarters. Core c -> bp = c//4
(batches {0,1} or {2,3}), uq = c%4 with u range [25*uq, 25*uq+26) (26 rows,
1-row overlap between quarters; quarter q>0 contributes local rows 1..25).
Per-core row space: 600 (b,t) rows laid out as 5 chunks of 120 partitions
with the two batches INTERLEAVED across partitions: chunk c, partition p ->
batch 2*bp + p%2, t = c*60 + p//2. This avoids any ragged 44-row t-tail
(no gpsimd elementwise), keeps DVE lane utilization at 94%, and keeps SDMA
engine load balanced.

The host supplies gelu inputs PRE-TRANSPOSED (encT/decT, feature dim on
partitions) -- gelu commutes with transpose -- so the kernel needs no PE
transposes; ACT gelus encT/decT in place and the matmuls read them directly
as lhsT. A burst of gapless N=128 matmuls during the input-load wait warms
the PE HAM clock-gate toward 2.4GHz before P_dec arrives. Per-u pipeline:
PE gathers the per-partition addend P_dec[batch(p), u] + bias straight out
of the [52,640] pd tile into a [120,640] f32 PSUM tile via a K=52 matmul
against a host-loaded one-hot selector slice selU[:, 128u:128u+120]; ACT
copies it to a bf16 bct tile; one DVE tensor_tensor (bf16 2x mode, ~1.8us)
adds pe[120,5,640] (P_enc, bf16) + bct (stride-0 broadcast over the 5
chunks) into the out tile j-slice, stored as one 768KB fully contiguous DMA
per u on alternating HWDGE rings (two in flight hides the ~2us HBM
write-receipt latency).

The first 4 u (blocks 0-1) instead go through a c-major early path so
stores start as soon as P_enc chunk 0 + pd are ready and flow while chunks
1-4 are still being computed: per-(chunk, u) DVE adds fill c-major otE
tiles whose chunk slices are stored immediately (307KB DMAs, 2560B
contiguous lines).
"""

import numpy as np

B, T, U = 4, 300, 101
D = 512
V = 640
UCORE = 26  # u rows per core (4 quarters x 25 + 1 overlap row)
NCORES = 8
UB = 2  # u rows per store block (26 = 13 * 2)
NBLK = UCORE // UB
NCH = 5  # row chunks of 120 partitions (600 = 2 batches x 300 t)
PCH = 120
NEARLY = 2  # leading blocks routed through the c-major early path
NWARM = 20  # gapless N=128 matmuls to open the PE HAM clock-gate

LAST_RESULT = None  # BassKernelResults of the most recent run (for test.py)
RUN_KWARGS = {}  # extra kwargs test.py may inject (e.g. tmpdir for traces)

_cache = {}


def _build():
    import concourse.mybir as mybir
    from concourse import bacc
    from concourse.tile import TileContext

    f32 = mybir.dt.float32
    bf16 = mybir.dt.bfloat16
    AF = mybir.ActivationFunctionType

    nc = bacc.Bacc()
    # host pre-transposed: encT[p, dch, r] = enc feature d = dch*128+p of row
    # r = c*120 + q  (row -> batch q%2... see module docstring)
    encT_d = nc.dram_tensor("encT", [128, 4, NCH * PCH], bf16, kind="ExternalInput")
    # decT[p, dch, s*26+u] = dec feature d = dch*128+p of (batch s, u0+u)
    decT_d = nc.dram_tensor("decT", [128, 4, 2 * UCORE], bf16, kind="ExternalInput")
    # W_fc.T rows d = dch*128+p; wTd = dec half (cols 512..1023), wTe = enc half
    wTd_d = nc.dram_tensor("wTd", [128, 4, V], bf16, kind="ExternalInput")
    wTe_d = nc.dram_tensor("wTe", [128, 4, V], bf16, kind="ExternalInput")
    bias_d = nc.dram_tensor("bias", [1, V], bf16, kind="ExternalInput")
    # selU[r, 128u + p] = 1 iff r == (p%2)*26 + u: K=52 gather-broadcast masks
    selU_d = nc.dram_tensor("selU", [2 * UCORE, UCORE * 128], bf16, kind="ExternalInput")
    # outputs laid out exactly like the SBUF tiles so every store is one
    # fully contiguous DRAM write; host un-permutes + upcasts to f32.
    # outE[b, c, p, j, v] = out[batch p%2, t = c*60 + p//2, u = UB*b + j, v]
    outE_d = nc.dram_tensor("outE", [NEARLY, NCH, PCH, UB, V], bf16, kind="ExternalOutput")
    # outD[bi, p, j, c, v] = out[.., u = NEARLY*UB + UB*bi + j, v]
    outD = nc.dram_tensor(
        "outD", [NBLK - NEARLY, PCH, UB, NCH, V], bf16, kind="ExternalOutput"
    )

    vchunks = [(0, 512), (512, V - 512)]

    with TileContext(nc) as tc:
        with (
            tc.tile_pool(name="const", bufs=1) as constp,
            tc.tile_pool(name="persist", bufs=1) as persist,
            tc.tile_pool(name="outp", bufs=5) as outp,
            tc.tile_pool(name="bctp", bufs=6) as bctp,
            tc.tile_pool(name="psum", bufs=1, space="PSUM") as psum,
        ):
            # input loads. sync ring: decT, wTd, bias (they gate the deepest
            # chain gelu -> P_dec -> bct). scalar ring: encT then selU (so
            # the ACT sequencer gets to its gelu table loads quickly). wTe on
            # the SWDGE ring overlaps both.
            decT = persist.tile([128, 4, 2 * UCORE], bf16, tag="decT")
            nc.sync.dma_start(decT[:, :, :], decT_d[:, :, :])
            wTd = persist.tile([128, 4, V], bf16, tag="wTd")
            for dch in range(4):
                nc.sync.dma_start(wTd[:, dch, :], wTd_d[:, dch, :])
            encT = persist.tile([128, 4, NCH * PCH], bf16, tag="encT")
            nc.scalar.dma_start(encT[:, :, :], encT_d[:, :, :])
            bias_sb = constp.tile([1, V], bf16)
            nc.sync.dma_start(bias_sb[:], bias_d[:])
            selU = constp.tile([2 * UCORE, UCORE * 128], bf16)
            nc.scalar.dma_start(selU[:, :], selU_d[:, :])
            ones1 = constp.tile([1, 128], bf16)
            nc.gpsimd.memset(ones1[:], 1.0)
            wTe = persist.tile([128, 4, V], bf16, tag="wTe")
            for dch in range(4):
                nc.gpsimd.dma_start(wTe[:, dch, :], wTe_d[:, dch, :])

            # warm the PE HAM clock-gate with gapless tiny matmuls while the
            # input DMAs land (~3.4us of sustained activity lifts the PE from
            # 1.2 to 2.4 GHz; the 4-deep psum rotation hides the WAW waits)
            for _ in range(NWARM):
                wm = psum.tile([128, V], f32, tag="bc", bufs=4)
                nc.tensor.matmul(wm[0:1, 0:128], ones1[0:1, 0:1], ones1[0:1, :], start=True, stop=True)

            # gelu in the transposed layout (gelu commutes with transpose)
            gdecT = persist.tile([128, 4, 2 * UCORE], bf16, tag="gdecT")
            nc.scalar.activation(gdecT[:, :, :], decT[:, :, :], AF.Gelu_apprx_tanh)
            gencT = persist.tile([128, 4, NCH * PCH], bf16, tag="gencT")
            nc.scalar.activation(gencT[:, :, :], encT[:, :, :], AF.Gelu_apprx_tanh)

            # P_dec + bias -> pd [52, 640] bf16 (kept partition-major; the
            # per-u broadcast gathers rows straight from it via selU).
            # d-major so matmuls start as soon as wTd lands.
            pd_bf = persist.tile([2 * UCORE, V], bf16, tag="pd")
            ps = psum.tile([128, V], f32, tag="bc", bufs=4)
            for d in range(4):
                for v0, vn in vchunks:
                    nc.tensor.matmul(
                        ps[: 2 * UCORE, v0 : v0 + vn],
                        gdecT[:, d, : 2 * UCORE],
                        wTd[:, d, v0 : v0 + vn],
                        start=(d == 0),
                        stop=False,
                    )
            for v0, vn in vchunks:
                nc.tensor.matmul(
                    ps[: 2 * UCORE, v0 : v0 + vn],
                    ones1[0:1, : 2 * UCORE],
                    bias_sb[:1, v0 : v0 + vn],
                    start=False,
                    stop=True,
                )
            nc.vector.tensor_copy(pd_bf[: 2 * UCORE, :], ps[: 2 * UCORE, :])

            pe = persist.tile([128, NCH, V], bf16, tag="pe")

            def enc_chunk(c):
                ps = psum.tile([128, V], f32, tag="bc", bufs=4)
                for d in range(4):
                    for v0, vn in vchunks:
                        nc.tensor.matmul(
                            ps[:PCH, v0 : v0 + vn],
                            gencT[:, d, c * PCH : (c + 1) * PCH],
                            wTe[:, d, v0 : v0 + vn],
                            start=(d == 0),
                            stop=(d == 3),
                        )
                nc.scalar.copy(pe[:PCH, c, :], ps[:PCH, :])

            def bcast_ps(u):
                ps = psum.tile([128, V], f32, tag="bc", bufs=4)
                for c0, cn in vchunks:
                    nc.tensor.matmul(
                        ps[:PCH, c0 : c0 + cn],
                        selU[:, u * 128 : u * 128 + PCH],
                        pd_bf[:, c0 : c0 + cn],
                        start=True,
                        stop=True,
                    )
                return ps

            # early path: blocks 0-1 (u 0..3) computed per chunk in c-major
            # ot tiles, each chunk slice stored (2560B contiguous lines) the
            # moment its DVE add lands -- stores flow while P_enc chunks 1-4
            # are still being computed.
            enc_chunk(0)
            bcts = []
            for u in range(NEARLY * UB):
                ps = bcast_ps(u)
                bct = bctp.tile([128, V], bf16, tag="bct", name="bct")
                nc.scalar.copy(bct[:PCH, :], ps[:PCH, :])
                bcts.append(bct)
            otE = [
                persist.tile([128, NCH, UB, V], bf16, tag=f"otE{b}", name=f"otE{b}")
                for b in range(NEARLY)
            ]
            for c in range(NCH):
                if c > 0:
                    enc_chunk(c)
                for b in range(NEARLY):
                    for j in range(UB):
                        nc.vector.tensor_add(
                            otE[b][:PCH, c, j, :],
                            pe[:PCH, c, :],
                            bcts[UB * b + j][:PCH, :],
                        )
                    eng = nc.sync if (c + b) % 2 == 0 else nc.scalar
                    eng.dma_start(outE_d[b, c, :, :, :], otE[b][:PCH, c, :, :])

            # main loop: blocks 4..12, one DVE add + one 768KB store per u,
            # stores alternating rings so two are always in flight
            for bi in range(NEARLY, NBLK):
                ot = outp.tile([128, UB, NCH, V], bf16, tag="ot", name="ot")
                for j in range(UB):
                    u = UB * bi + j
                    ps = bcast_ps(u)
                    bct = bctp.tile([128, V], bf16, tag="bct", name="bct")
                    nc.scalar.copy(bct[:PCH, :], ps[:PCH, :])
                    nc.vector.tensor_add(
                        ot[:PCH, j, :, :],
                        pe[:PCH, :, :],
                        bct[:PCH, :].unsqueeze(1).broadcast_to([PCH, NCH, V]),
                    )
                    eng = nc.sync if j % 2 == 0 else nc.scalar
                    eng.dma_start(outD[bi - NEARLY, :, j, :, :], ot[:PCH, j, :, :])

    nc.compile()
    return nc


def kernel(encoder_outputs, decoder_outputs, W_fc, b_fc):
    global LAST_RESULT
    import os

    import ml_dtypes
    from concourse.bass_utils import run_bass_kernel_spmd

    bf = ml_dtypes.bfloat16
    enc = np.asarray(encoder_outputs, dtype=np.float32)
    dec = np.asarray(decoder_outputs, dtype=np.float32)

    # row space per batch-pair: r = c*120 + 2*i + s -> batch 2*bp+s, t = c*60+i
    E = enc.reshape(2, 2, NCH, 60, D).transpose(0, 2, 3, 1, 4).reshape(2, 600, D)
    # encT[bp] = E[bp].T reshaped to [128, 4, 600]
    encT = np.ascontiguousarray(
        E.transpose(0, 2, 1).reshape(2, 4, 128, NCH * PCH).transpose(0, 2, 1, 3)
    ).astype(bf)

    wT = np.asarray(W_fc, dtype=np.float32).T  # (1024, 640)
    wT_tiled = wT.reshape(8, 128, V).transpose(1, 0, 2)  # [128, 8, 640]
    wTe = np.ascontiguousarray(wT_tiled[:, 0:4]).astype(bf)
    wTd = np.ascontiguousarray(wT_tiled[:, 4:8]).astype(bf)

    bias = np.asarray(b_fc, dtype=np.float32)[None, :].astype(bf)

    # selU[r, 128u + p] = 1 iff r == (p%2)*26 + u
    selU = np.zeros((2 * UCORE, UCORE * 128), dtype=bf)
    for u in range(UCORE):
        selU[u, u * 128 + 0 : (u + 1) * 128 : 2] = 1
        selU[UCORE + u, u * 128 + 1 : (u + 1) * 128 : 2] = 1

    if "nc" not in _cache:
        _cache["nc"] = _build()
    nc = _cache["nc"]

    in_maps = []
    for c in range(NCORES):
        bp, uq = c // 4, c % 4
        u0 = 25 * uq
        # Dc[s*26+u, :] = dec[2*bp+s, u0+u, :]; decT = Dc.T as [128, 4, 52]
        Dc = dec[2 * bp : 2 * bp + 2, u0 : u0 + UCORE].reshape(2 * UCORE, D)
        decT = np.ascontiguousarray(
            Dc.T.reshape(4, 128, 2 * UCORE).transpose(1, 0, 2)
        ).astype(bf)
        in_maps.append(
            {
                "encT": encT[bp],
                "decT": decT,
                "wTd": wTd,
                "wTe": wTe,
                "bias": bias,
                "selU": selU,
            }
        )

    res = run_bass_kernel_spmd(
        nc,
        in_maps,
        list(range(NCORES)),
        trace=bool(int(os.environ.get("KJ_TRACE", "0"))),
        **RUN_KWARGS,
    )
    LAST_RESULT = res

    out = np.empty((B, T, U, V), dtype=np.float32)
    for c in range(NCORES):
        bp, uq = c // 4, c % 4
        u0 = 25 * uq
        # bf16 -> f32 exact upcast via bit shift (fast)
        aE = res.results[c]["outE"]  # [2, 5, 120, 2, 640] -> u 0..3
        aD = res.results[c]["outD"]  # [11, 120, 2, 5, 640] -> u 4..25
        fE = (aE.view(np.uint16).astype(np.uint32) << 16).view(np.float32)
        fD = (aD.view(np.uint16).astype(np.uint32) << 16).view(np.float32)
        # outE [b, cc, p, j, v]: p -> (i, s); -> [s, cc, i, b, j, v]
        cutE = np.ascontiguousarray(
            fE.reshape(NEARLY, NCH, 60, 2, UB, V).transpose(3, 1, 2, 0, 4, 5)
        ).reshape(2, T, NEARLY * UB, V)
        # outD [bi, p, j, cc, v] -> [s, cc, i, bi, j, v]
        cutD = np.ascontiguousarray(
            fD.reshape(NBLK - NEARLY, 60, 2, UB, NCH, V).transpose(2, 4, 1, 0, 3, 5)
        ).reshape(2, T, UCORE - NEARLY * UB, V)
        cut = np.concatenate([cutE, cutD], axis=2)  # (2, 300, 26, 640)
        lo = 0 if uq == 0 else 1  # quarter q>0: local row 0 is the overlap
        out[2 * bp, :, u0 + lo : u0 + UCORE, :] = cut[0, :, lo:, :]
        out[2 * bp + 1, :, u0 + lo : u0 + UCORE, :] = cut[1, :, lo:, :]
    return out


# revision 25
# speedup vs baseline: 1.0754x; 1.0258x over previous
"""RNN-T JointNet kernel for 8 Trainium2 NeuronCores.

Math: out[b,t,u,:] = gelu_tanh(concat(enc[b,t], dec[b,u])) @ W_fc^T + b_fc
Since gelu is elementwise, gelu(concat(a,b)) = concat(gelu(a), gelu(b)), so
  out[b,t,u,:] = P_enc[b,t,:] + P_dec[b,u,:]
with P_enc = gelu(enc) @ W_fc[:, :512]^T          (small matmul)
     P_dec = gelu(dec) @ W_fc[:, 512:]^T + b_fc   (small matmul; bias folded
                                                   here so it rides the bct)
The dominant cost is streaming the (B,T,U,V) output to HBM. The output is
stored as bf16 (compute is already bf16; rel err ~4e-3) and upcast to f32 on
the host, halving HBM store traffic vs f32 (~20MB/core, ~55us at ~360GB/s).

Sharding: 8 cores = 2 batch-pairs x 4 u-quarters. Core c -> bp = c//4
(batches {0,1} or {2,3}), uq = c%4 with u range [25*uq, 25*uq+26) (26 rows,
1-row overlap between quarters; quarter q>0 contributes local rows 1..25).
Per-core row space: 600 (b,t) rows laid out as 5 chunks of 120 partitions
with the two batches INTERLEAVED across partitions: chunk c, partition p ->
batch 2*bp + p%2, t = c*60 + p//2. This avoids any ragged 44-row t-tail
(no gpsimd elementwise), keeps DVE lane utilization at 94%, and keeps SDMA
engine load balanced.

The host supplies gelu inputs PRE-TRANSPOSED (encT/decT, feature dim on
partitions) -- gelu commutes with transpose -- so the kernel needs no PE
transposes; ACT gelus encT/decT in place and the matmuls read them directly
as lhsT. A burst of gapless N=128 matmuls during the input-load wait warms
the PE HAM clock-gate toward 2.4GHz before P_dec arrives. Per-u pipeline:
PE gathers the per-partition addend P_dec[batch(p), u] + bias straight out
of the [52,640] pd tile into a [120,640] f32 PSUM tile via a K=52 matmul
against a host-loaded one-hot selector slice selU[:, 128u:128u+120]; ACT
copies it to a bf16 bct tile; one DVE tensor_tensor (bf16 2x mode, ~1.8us)
adds pe[120,5,640] (P_enc, bf16) + bct (stride-0 broadcast over the 5
chunks) into the out tile j-slice, stored as one 768KB fully contiguous DMA
per u on alternating HWDGE rings (two in flight hides the ~2us HBM
write-receipt latency).

The first 4 u (blocks 0-1) instead go through a c-major early path so
stores start as soon as P_enc chunk 0 + pd are ready and flow while chunks
1-4 are still being computed: per-(chunk, u) DVE adds fill c-major otE
tiles whose chunk slices are stored immediately (307KB DMAs, 2560B
contiguous lines).
"""

import numpy as np

B, T, U = 4, 300, 101
D = 512
V = 640
UCORE = 26  # u rows per core (4 quarters x 25 + 1 overlap row)
NCORES = 8
UB = 2  # u rows per store block (26 = 13 * 2)
NBLK = UCORE // UB
NCH = 5  # row chunks of 120 partitions (600 = 2 batches x 300 t)
PCH = 120
NEARLY = 2  # leading blocks routed through the c-major early path
NWARM = 20  # gapless N=128 matmuls to open the PE HAM clock-gate

LAST_RESULT = None  # BassKernelResults of the most recent run (for test.py)
RUN_KWARGS = {}  # extra kwargs test.py may inject (e.g. tmpdir for traces)

_cache = {}


def _build():
    import concourse.mybir as mybir
    from concourse import bacc
    from concourse.tile import TileContext

    f32 = mybir.dt.float32
    bf16 = mybir.dt.bfloat16
    AF = mybir.ActivationFunctionType

    nc = bacc.Bacc()
    # host pre-transposed: encT[p, dch, r] = enc feature d = dch*128+p of row
    # r = c*120 + q  (row -> batch q%2... see module docstring)
    encT_d = nc.dram_tensor("encT", [128, 4, NCH * PCH], bf16, kind="ExternalInput")
    # decT[p, dch, s*26+u] = dec feature d = dch*128+p of (batch s, u0+u)
    decT_d = nc.dram_tensor("decT", [128, 4, 2 * UCORE], bf16, kind="ExternalInput")
    # W_fc.T rows d = dch*128+p; wTd = dec half (cols 512..1023), wTe = enc half
    wTd_d = nc.dram_tensor("wTd", [128, 4, V], bf16, kind="ExternalInput")
    wTe_d = nc.dram_tensor("wTe", [128, 4, V], bf16, kind="ExternalInput")
    bias_d = nc.dram_tensor("bias", [1, V], bf16, kind="ExternalInput")
    # selU[r, 128u + p] = 1 iff r == (p%2)*26 + u: K=52 gather-broadcast masks
    selU_d = nc.dram_tensor("selU", [2 * UCORE, UCORE * 128], bf16, kind="ExternalInput")
    # outputs laid out exactly like the SBUF tiles so every store is one
    # fully contiguous DRAM write; host un-permutes + upcasts to f32.
    # outE[b, c, p, j, v] = out[batch p%2, t = c*60 + p//2, u = UB*b + j, v]
    outE_d = nc.dram_tensor("outE", [NEARLY, NCH, PCH, UB, V], bf16, kind="ExternalOutput")
    # outD[bi, p, j, c, v] = out[.., u = NEARLY*UB + UB*bi + j, v]
    outD = nc.dram_tensor(
        "outD", [NBLK - NEARLY, PCH, UB, NCH, V], bf16, kind="ExternalOutput"
    )

    vchunks = [(0, 512), (512, V - 512)]

    with TileContext(nc) as tc:
        with (
            tc.tile_pool(name="const", bufs=1) as constp,
            tc.tile_pool(name="persist", bufs=1) as persist,
            tc.tile_pool(name="outp", bufs=5) as outp,
            tc.tile_pool(name="bctp", bufs=6) as bctp,
            tc.tile_pool(name="psum", bufs=1, space="PSUM") as psum,
        ):
            # input loads. sync ring: decT, wTd, bias (they gate the deepest
            # chain gelu -> P_dec -> bct). scalar ring: encT then selU (so
            # the ACT sequencer gets to its gelu table loads quickly). wTe on
            # the SWDGE ring overlaps both.
            # Input load schedule exploits per-ring FIFO ordering as a
            # PRIORITY mechanism: the sync ring serializes bias -> decT ->
            # wTd halves -> wTe, so the P_dec weights stream ahead of the
            # P_enc weights while encT runs in parallel on the scalar ring.
            # selU's issue is placed between the two gelus on the ACT
            # sequencer so its 346KB don't steal bandwidth from the
            # critical wTd/encT window.
            bias_sb = constp.tile([1, V], bf16)
            nc.sync.dma_start(bias_sb[:], bias_d[:])
            decT = persist.tile([128, 4, 2 * UCORE], bf16, tag="decT")
            nc.sync.dma_start(decT[:, :, :], decT_d[:, :, :])
            wTd = persist.tile([128, 4, V], bf16, tag="wTd")
            nc.sync.dma_start(wTd[:, 0:2, :], wTd_d[:, 0:2, :])
            nc.sync.dma_start(wTd[:, 2:4, :], wTd_d[:, 2:4, :])
            wTe = persist.tile([128, 4, V], bf16, tag="wTe")
            nc.sync.dma_start(wTe[:, :, :], wTe_d[:, :, :])
            encT = persist.tile([128, 4, NCH * PCH], bf16, tag="encT")
            nc.scalar.dma_start(encT[:, :, :], encT_d[:, :, :])
            ones1 = constp.tile([1, 128], bf16)
            nc.gpsimd.memset(ones1[:], 1.0)

            # warm the PE HAM clock-gate with gapless tiny matmuls while the
            # input DMAs land (~3.4us of sustained activity lifts the PE from
            # 1.2 to 2.4 GHz; the 4-deep psum rotation hides the WAW waits)
            for _ in range(NWARM):
                wm = psum.tile([128, V], f32, tag="bc", bufs=4)
                nc.tensor.matmul(wm[0:1, 0:128], ones1[0:1, 0:1], ones1[0:1, :], start=True, stop=True)

            # gelu in the transposed layout (gelu commutes with transpose)
            gdecT = persist.tile([128, 4, 2 * UCORE], bf16, tag="gdecT")
            nc.scalar.activation(gdecT[:, :, :], decT[:, :, :], AF.Gelu_apprx_tanh)
            selU = constp.tile([2 * UCORE, UCORE * 128], bf16)
            nc.scalar.dma_start(selU[:, :], selU_d[:, :])
            gencT = persist.tile([128, 4, NCH * PCH], bf16, tag="gencT")
            nc.scalar.activation(gencT[:, :, :], encT[:, :, :], AF.Gelu_apprx_tanh)

            # P_dec + bias -> pd [52, 640] bf16 (kept partition-major; the
            # per-u broadcast gathers rows straight from it via selU).
            # d-major so matmuls start as soon as wTd lands.
            pd_bf = persist.tile([2 * UCORE, V], bf16, tag="pd")
            ps = psum.tile([128, V], f32, tag="bc", bufs=4)
            for d in range(4):
                for v0, vn in vchunks:
                    nc.tensor.matmul(
                        ps[: 2 * UCORE, v0 : v0 + vn],
                        gdecT[:, d, : 2 * UCORE],
                        wTd[:, d, v0 : v0 + vn],
                        start=(d == 0),
                        stop=False,
                    )
            for v0, vn in vchunks:
                nc.tensor.matmul(
                    ps[: 2 * UCORE, v0 : v0 + vn],
                    ones1[0:1, : 2 * UCORE],
                    bias_sb[:1, v0 : v0 + vn],
                    start=False,
                    stop=True,
                )
            nc.vector.tensor_copy(pd_bf[: 2 * UCORE, :], ps[: 2 * UCORE, :])

            pe = persist.tile([128, NCH, V], bf16, tag="pe")

            def enc_chunk(c):
                ps = psum.tile([128, V], f32, tag="bc", bufs=4)
                for d in range(4):
                    for v0, vn in vchunks:
                        nc.tensor.matmul(
                            ps[:PCH, v0 : v0 + vn],
                            gencT[:, d, c * PCH : (c + 1) * PCH],
                            wTe[:, d, v0 : v0 + vn],
                            start=(d == 0),
                            stop=(d == 3),
                        )
                nc.scalar.copy(pe[:PCH, c, :], ps[:PCH, :])

            def bcast_ps(u):
                ps = psum.tile([128, V], f32, tag="bc", bufs=4)
                for c0, cn in vchunks:
                    nc.tensor.matmul(
                        ps[:PCH, c0 : c0 + cn],
                        selU[:, u * 128 : u * 128 + PCH],
                        pd_bf[:, c0 : c0 + cn],
                        start=True,
                        stop=True,
                    )
                return ps

            # early path: blocks 0-1 (u 0..3) computed per chunk in c-major
            # ot tiles, each chunk slice stored (2560B contiguous lines) the
            # moment its DVE add lands -- stores flow while P_enc chunks 1-4
            # are still being computed.
            enc_chunk(0)
            bcts = []
            for u in range(NEARLY * UB):
                ps = bcast_ps(u)
                bct = bctp.tile([128, V], bf16, tag="bct", name="bct")
                nc.scalar.copy(bct[:PCH, :], ps[:PCH, :])
                bcts.append(bct)
            otE = [
                persist.tile([128, NCH, UB, V], bf16, tag=f"otE{b}", name=f"otE{b}")
                for b in range(NEARLY)
            ]
            for c in range(NCH):
                if c > 0:
                    enc_chunk(c)
                for b in range(NEARLY):
                    for j in range(UB):
                        nc.vector.tensor_add(
                            otE[b][:PCH, c, j, :],
                            pe[:PCH, c, :],
                            bcts[UB * b + j][:PCH, :],
                        )
                    eng = nc.sync if (c + b) % 2 == 0 else nc.scalar
                    eng.dma_start(outE_d[b, c, :, :, :], otE[b][:PCH, c, :, :])

            # main loop: blocks 4..12, one DVE add + one 768KB store per u,
            # stores alternating rings so two are always in flight
            for bi in range(NEARLY, NBLK):
                ot = outp.tile([128, UB, NCH, V], bf16, tag="ot", name="ot")
                for j in range(UB):
                    u = UB * bi + j
                    ps = bcast_ps(u)
                    bct = bctp.tile([128, V], bf16, tag="bct", name="bct")
                    nc.scalar.copy(bct[:PCH, :], ps[:PCH, :])
                    nc.vector.tensor_add(
                        ot[:PCH, j, :, :],
                        pe[:PCH, :, :],
                        bct[:PCH, :].unsqueeze(1).broadcast_to([PCH, NCH, V]),
                    )
                    eng = nc.sync if j % 2 == 0 else nc.scalar
                    eng.dma_start(outD[bi - NEARLY, :, j, :, :], ot[:PCH, j, :, :])

    nc.compile()
    return nc


def kernel(encoder_outputs, decoder_outputs, W_fc, b_fc):
    global LAST_RESULT
    import os

    import ml_dtypes
    from concourse.bass_utils import run_bass_kernel_spmd

    bf = ml_dtypes.bfloat16
    enc = np.asarray(encoder_outputs, dtype=np.float32)
    dec = np.asarray(decoder_outputs, dtype=np.float32)

    # row space per batch-pair: r = c*120 + 2*i + s -> batch 2*bp+s, t = c*60+i
    E = enc.reshape(2, 2, NCH, 60, D).transpose(0, 2, 3, 1, 4).reshape(2, 600, D)
    # encT[bp] = E[bp].T reshaped to [128, 4, 600]
    encT = np.ascontiguousarray(
        E.transpose(0, 2, 1).reshape(2, 4, 128, NCH * PCH).transpose(0, 2, 1, 3)
    ).astype(bf)

    wT = np.asarray(W_fc, dtype=np.float32).T  # (1024, 640)
    wT_tiled = wT.reshape(8, 128, V).transpose(1, 0, 2)  # [128, 8, 640]
    wTe = np.ascontiguousarray(wT_tiled[:, 0:4]).astype(bf)
    wTd = np.ascontiguousarray(wT_tiled[:, 4:8]).astype(bf)

    bias = np.asarray(b_fc, dtype=np.float32)[None, :].astype(bf)

    # selU[r, 128u + p] = 1 iff r == (p%2)*26 + u
    selU = np.zeros((2 * UCORE, UCORE * 128), dtype=bf)
    for u in range(UCORE):
        selU[u, u * 128 + 0 : (u + 1) * 128 : 2] = 1
        selU[UCORE + u, u * 128 + 1 : (u + 1) * 128 : 2] = 1

    if "nc" not in _cache:
        _cache["nc"] = _build()
    nc = _cache["nc"]

    in_maps = []
    for c in range(NCORES):
        bp, uq = c // 4, c % 4
        u0 = 25 * uq
        # Dc[s*26+u, :] = dec[2*bp+s, u0+u, :]; decT = Dc.T as [128, 4, 52]
        Dc = dec[2 * bp : 2 * bp + 2, u0 : u0 + UCORE].reshape(2 * UCORE, D)
        decT = np.ascontiguousarray(
            Dc.T.reshape(4, 128, 2 * UCORE).transpose(1, 0, 2)
        ).astype(bf)
        in_maps.append(
            {
                "encT": encT[bp],
                "decT": decT,
                "wTd": wTd,
                "wTe": wTe,
                "bias": bias,
                "selU": selU,
            }
        )

    res = run_bass_kernel_spmd(
        nc,
        in_maps,
        list(range(NCORES)),
        trace=bool(int(os.environ.get("KJ_TRACE", "0"))),
        **RUN_KWARGS,
    )
    LAST_RESULT = res

    out = np.empty((B, T, U, V), dtype=np.float32)
    for c in range(NCORES):
        bp, uq = c // 4, c % 4
        u0 = 25 * uq
        # bf16 -> f32 exact upcast via bit shift (fast)
        aE = res.results[c]["outE"]  # [2, 5, 120, 2, 640] -> u 0..3
        aD = res.results[c]["outD"]  # [11, 120, 2, 5, 640] -> u 4..25
        fE = (aE.view(np.uint16).astype(np.uint32) << 16).view(np.float32)
        fD = (aD.view(np.uint16).astype(np.uint32) << 16).view(np.float32)
        # outE [b, cc, p, j, v]: p -> (i, s); -> [s, cc, i, b, j, v]
        cutE = np.ascontiguousarray(
            fE.reshape(NEARLY, NCH, 60, 2, UB, V).transpose(3, 1, 2, 0, 4, 5)
        ).reshape(2, T, NEARLY * UB, V)
        # outD [bi, p, j, cc, v] -> [s, cc, i, bi, j, v]
        cutD = np.ascontiguousarray(
            fD.reshape(NBLK - NEARLY, 60, 2, UB, NCH, V).transpose(2, 4, 1, 0, 3, 5)
        ).reshape(2, T, UCORE - NEARLY * UB, V)
        cut = np.concatenate([cutE, cutD], axis=2)  # (2, 300, 26, 640)
        lo = 0 if uq == 0 else 1  # quarter q>0: local row 0 is the overlap
        out[2 * bp, :, u0 + lo : u0 + UCORE, :] = cut[0, :, lo:, :]
        out[2 * bp + 1, :, u0 + lo : u0 + UCORE, :] = cut[1, :, lo:, :]
    return out
